# revision 1
# baseline (speedup 1.0000x reference)
"""Trainium2 Bass kernel for nn_ExtractorMLP (GNN edge cosine-similarity logits).

Math: out[e] = cos(MLP(emb[col[e]]), MLP(emb[row[e]])) for E edges, where
MLP(x) = relu(x @ W1.T + b1) @ W2.T + b2, cos uses torch eps=1e-8 semantics.

Strategy (8 cores, SPMD, identical program, per-core edge shards):
  Phase 1 (replicated): run the MLP over ALL N nodes once per core
    (12x fewer FLOPs than per-edge MLP), normalize each output row,
    store a bf16 table gn[N, H] in core-local DRAM.
    Layouts avoid every transpose: L1 computes H1T (h-major) with
    lhsT=W1.T tiles; L2 uses lhsT=H1T chunks to come back to n-major.
  Phase 2 (edge shard, E/8 per core): chunked transposed dma_gather of
    gn rows for col/row endpoints (H across partitions, edges along the
    free dim), bf16 elementwise multiply on DVE, then a ones-vector
    matmul on PE reduces across partitions to per-edge dots in PSUM,
    DMA'd straight to the output. dma_gather uses int16 indices, so
    edges are grouped host-side into 4 groups by (col<32768, row<32768)
    and gathered against per-half table base offsets.
"""

import sys

for _p in ("/opt/trn_rl_repo",):
    if _p not in sys.path:
        sys.path.insert(0, _p)

import numpy as np
import ml_dtypes

import concourse.bass as bass
import concourse.bacc as bacc
import concourse.mybir as mybir
import concourse.tile as tile
from concourse.tile import add_dep_helper
from concourse.bass_utils import run_bass_kernel_spmd

BF16 = mybir.dt.bfloat16
F32 = mybir.dt.float32
I16 = mybir.dt.int16

# Problem sizes (hardcoded per harness contract)
N, H, E = 50000, 256, 300000
NCORES = 8
F = 512                          # node-phase free-dim block (nodes per block)
NPAD = ((N + F - 1) // F) * F    # 50176
NBLK = NPAD // F                 # 98
EPC = E // NCORES                # 37500 edges per core
HALF = 32768                     # int16 index range split point
GCHUNK = 4096                    # edges per dma_gather


def _group_caps(epc):
    """Compile-time per-group capacities: mean + 8 sigma, rounded to 128."""
    p = HALF / N
    probs = [p * p, p * (1 - p), (1 - p) * p, (1 - p) * (1 - p)]
    caps = []
    for pr in probs:
        mean = epc * pr
        sig = (epc * pr * (1 - pr)) ** 0.5
        caps.append(int(np.ceil((mean + 8 * sig) / 128)) * 128)
    return caps


GCAPS = _group_caps(EPC)         # [16896, 9216, 9216, 5120]
GOFFS = [int(x) for x in np.cumsum([0] + GCAPS[:-1])]
TOTE = sum(GCAPS)                # 40448


def build_bass(n_pad, n_blk, f, gcaps, half, gchunk, table_dt=BF16):
    """Build the SPMD Bass module. Parametrized for small-scale sim tests."""
    nc = bacc.Bacc("TRN2", target_bir_lowering=False, num_swdge_queues=4)
    h = H
    tote = sum(gcaps)
    goffs = [int(x) for x in np.cumsum([0] + list(gcaps[:-1]))]

    embT = nc.dram_tensor("embT", [h, n_pad], BF16, kind="ExternalInput")
    w1t = nc.dram_tensor("w1t", [h, h], BF16, kind="ExternalInput")
    w2t = nc.dram_tensor("w2t", [h, h], BF16, kind="ExternalInput")
    b1c = nc.dram_tensor("b1c", [h, 1], F32, kind="ExternalInput")
    b2rb = nc.dram_tensor("b2rb", [1, h], BF16, kind="ExternalInput")
    colw = nc.dram_tensor("colw", [128, tote // 16], I16, kind="ExternalInput")
    roww = nc.dram_tensor("roww", [128, tote // 16], I16, kind="ExternalInput")
    dots_out = nc.dram_tensor("dots", [128, tote // 128], F32, kind="ExternalOutput")
    gn = nc.dram_tensor("gn_table", [n_pad, h], table_dt)  # internal

    AF = mybir.ActivationFunctionType
    OP = mybir.AluOpType
    AX = mybir.AxisListType

    with tile.TileContext(nc) as tc:
        with (
            tc.tile_pool(name="const", bufs=1) as constp,
            tc.tile_pool(name="xt", bufs=4) as xtp,
            tc.tile_pool(name="h1", bufs=3) as h1p,
            tc.tile_pool(name="gg", bufs=4) as gp,
            tc.tile_pool(name="small", bufs=4) as sp,
            tc.tile_pool(name="ps1", bufs=4, space="PSUM") as ps1,
            tc.tile_pool(name="ps2", bufs=2, space="PSUM") as ps2,
            tc.tile_pool(name="ebuf", bufs=2) as ep,
        ):
            # ---- constants ----
            w1k = []
            w2k = []
            b1t = []
            for k in range(2):
                t_ = constp.tile([128, h], BF16, tag=f"w1_{k}")
                nc.sync.dma_start(out=t_[:], in_=w1t[k * 128:(k + 1) * 128, :])
                w1k.append(t_)
                t_ = constp.tile([128, h], BF16, tag=f"w2_{k}")
                nc.sync.dma_start(out=t_[:], in_=w2t[k * 128:(k + 1) * 128, :])
                w2k.append(t_)
                t_ = constp.tile([128, 1], F32, tag=f"b1_{k}")
                nc.sync.dma_start(out=t_[:], in_=b1c[k * 128:(k + 1) * 128, :])
                b1t.append(t_)
            b2row = constp.tile([1, h], BF16, tag="b2row")
            nc.sync.dma_start(out=b2row[:], in_=b2rb[:])
            ones_row = constp.tile([1, 128], BF16, tag="ones_row")
            nc.vector.memset(ones_row[:], 1.0)
            colsb = constp.tile([128, tote // 16], I16, tag="colsb")
            nc.sync.dma_start(out=colsb[:], in_=colw[:])
            rowsb = constp.tile([128, tote // 16], I16, tag="rowsb")
            nc.sync.dma_start(out=rowsb[:], in_=roww[:])

            # ---- phase 1: node MLP -> normalized bf16 table ----
            for b in range(n_blk):
                n0 = b * f
                xtk = []
                for k in range(2):
                    t_ = xtp.tile([128, f], BF16, tag=f"xt{k}")
                    nc.sync.dma_start(
                        out=t_[:], in_=embT[k * 128:(k + 1) * 128, n0:n0 + f]
                    )
                    xtk.append(t_)
                h1 = []
                for t in range(2):
                    p1 = ps1.tile([128, f], F32, tag="p1")
                    for k in range(2):
                        nc.tensor.matmul(
                            p1[:],
                            lhsT=w1k[k][:, t * 128:(t + 1) * 128],
                            rhs=xtk[k][:],
                            start=(k == 0),
                            stop=(k == 1),
                        )
                    ht = h1p.tile([128, f], BF16, tag=f"h1_{t}")
                    nc.scalar.activation(ht[:], p1[:], AF.Relu, bias=b1t[t][:])
                    h1.append(ht)
                nch = f // 128
                p2b = ps2.tile([128, nch, h], F32, tag="p2")
                for c in range(nch):
                    for t in range(2):
                        nc.tensor.matmul(
                            p2b[:, c, :],
                            lhsT=h1[t][:, c * 128:(c + 1) * 128],
                            rhs=w2k[t][:],
                            start=(t == 0),
                            stop=False,
                        )
                    # + b2 broadcast via rank-1 ones matmul (PE, not DVE)
                    nc.tensor.matmul(
                        p2b[:, c, :],
                        lhsT=ones_row[:],
                        rhs=b2row[:],
                        start=False,
                        stop=True,
                    )
                # norms^2: ACT squares the whole block, DVE does a 3D reduce
                sqb = gp.tile([128, nch, h], F32, tag="sqb")
                nc.scalar.activation(sqb[:], p2b[:], AF.Square)
                n2 = sp.tile([128, nch], F32, tag="n2")
                nc.vector.tensor_reduce(
                    out=n2[:], in_=sqb[:], axis=AX.X, op=OP.add,
                )
                s_ = sp.tile([128, nch], F32, tag="s")
                nc.scalar.activation(s_[:], n2[:], AF.Sqrt)
                sm = sp.tile([128, nch], F32, tag="sm")
                nc.vector.tensor_scalar_max(sm[:], s_[:], 1e-8)
                inv = sp.tile([128, nch], F32, tag="inv")
                nc.vector.reciprocal(inv[:], sm[:])
                gnb = gp.tile([128, nch, h], table_dt, tag="gnb")
                nc.vector.tensor_tensor(
                    out=gnb[:], in0=p2b[:],
                    in1=inv[:].to_broadcast([128, nch, h]), op=OP.mult,
                )
                nc.sync.dma_start(
                    out=gn[n0:n0 + f, :].rearrange("(c p) h -> p c h", p=128),
                    in_=gnb[:],
                )

            # all table writes must land before any gather
            tc.strict_bb_all_engine_barrier()

            # ---- phase 2: grouped gathers (4 SWDGE queues) + dots ----
            bases = [
                (0, 0), (0, half), (half, 0), (half, half)
            ]  # (col base, row base) per group
            dots = constp.tile([128, tote // 128], F32, tag="dots")
            qi = 0
            prev_gather = None
            for g in range(4):
                cb, rb = bases[g]
                cb = cb if cb < n_pad else 0  # small-config: high groups empty
                rb = rb if rb < n_pad else 0
                src_c = gn[cb:, :] if cb else gn[:]
                src_r = gn[rb:, :] if rb else gn[:]
                for c0 in range(0, gcaps[g], gchunk):
                    nI = min(gchunk, gcaps[g] - c0)
                    nb = nI // 128
                    w0 = (goffs[g] + c0) // 16
                    g1 = ep.tile([128, nb, h], table_dt, tag="g1")
                    g2 = ep.tile([128, nb, h], table_dt, tag="g2")
                    gi1 = nc.gpsimd.dma_gather(
                        g1[:], src_c, colsb[:, w0:w0 + nI // 16],
                        nI, nI, h, transpose=False, single_packet=False,
                        queue_num=qi % 4,
                    )
                    qi += 1
                    gi2 = nc.gpsimd.dma_gather(
                        g2[:], src_r, rowsb[:, w0:w0 + nI // 16],
                        nI, nI, h, transpose=False, single_packet=False,
                        queue_num=qi % 4,
                    )
                    qi += 1
                    # pin scheduler order so DMASW lane rotation stays
                    # aligned with the queue_num stripe (lane i%8 <-> queue i%4)
                    if prev_gather is not None:
                        add_dep_helper(gi1.ins, prev_gather.ins, sync=False,
                                       reason="swdge lane/queue alignment")
                    add_dep_helper(gi2.ins, gi1.ins, sync=False,
                                   reason="swdge lane/queue alignment")
                    prev_gather = gi2
                    prod = ep.tile([128, nb, h], table_dt, tag="prod")
                    nc.vector.tensor_tensor(
                        out=prod[:], in0=g1[:], in1=g2[:], op=OP.mult,
                    )
                    b0 = (goffs[g] + c0) // 128
                    nc.vector.tensor_reduce(
                        out=dots[:, b0:b0 + nb], in_=prod[:], axis=AX.X, op=OP.add,
                    )
            nc.sync.dma_start(out=dots_out[:], in_=dots[:])

    return nc


def make_inputs(emb, W1, b1, W2, b2, col, row, n_pad, gcaps, ncores):
    """Host-side prep: transposes, bf16 rounding, per-core group shards.

    Returns (in_maps, scatter) where scatter[c] = (positions, goffs_lens)
    for reassembling per-core outputs.
    """
    h = emb.shape[1]
    embT = np.zeros((h, n_pad), dtype=ml_dtypes.bfloat16)
    embT[:, :emb.shape[0]] = emb.astype(ml_dtypes.bfloat16).T
    w1t = np.ascontiguousarray(W1.astype(ml_dtypes.bfloat16).T)
    w2t = np.ascontiguousarray(W2.astype(ml_dtypes.bfloat16).T)
    b1c = np.ascontiguousarray(b1.astype(np.float32).reshape(h, 1))
    b2rb = b2.astype(ml_dtypes.bfloat16).reshape(1, h)
    epc = len(col) // ncores
    goffs = [int(x) for x in np.cumsum([0] + list(gcaps[:-1]))]
    tote = sum(gcaps)

    def wrap16(a):
        return np.tile(a.reshape(-1, 16).T, (8, 1)).astype(np.int16)

    in_maps = []
    scatter = []
    for c in range(ncores):
        cs = col[c * epc:(c + 1) * epc].astype(np.int64)
        rs = row[c * epc:(c + 1) * epc].astype(np.int64)
        gid = (cs >= HALF) * 2 + (rs >= HALF)
        colw = np.zeros(tote, dtype=np.int16)
        roww = np.zeros(tote, dtype=np.int16)
        positions = []
        lens = []
        for g in range(4):
            pos = np.nonzero(gid == g)[0]
            pos = pos[np.argsort(cs[pos], kind="stable")]
            ng = len(pos)
            assert ng <= gcaps[g], f"group {g} overflow: {ng} > {gcaps[g]}"
            cb = HALF if g >= 2 else 0
            rb = HALF if g % 2 else 0
            colw[goffs[g]:goffs[g] + ng] = (cs[pos] - cb).astype(np.int16)
            roww[goffs[g]:goffs[g] + ng] = (rs[pos] - rb).astype(np.int16)
            positions.append(pos)
            lens.append(ng)
        in_maps.append({
            "embT": embT, "w1t": w1t, "w2t": w2t, "b1c": b1c, "b2rb": b2rb,
            "colw": wrap16(colw), "roww": wrap16(roww),
        })
        scatter.append((positions, lens))
    return in_maps, scatter


def unshard_output(outs, scatter, gcaps, epc, ncores):
    goffs = [int(x) for x in np.cumsum([0] + list(gcaps[:-1]))]
    parts = []
    for c in range(ncores):
        dots = np.asarray(outs[c]["dots"]).T.reshape(-1)
        positions, lens = scatter[c]
        res = np.empty(epc, dtype=np.float32)
        for g in range(4):
            res[positions[g]] = dots[goffs[g]:goffs[g] + lens[g]]
        parts.append(res)
    return np.concatenate(parts)


_NC_CACHE = {}


def get_nc():
    if "nc" not in _NC_CACHE:
        nc_ = build_bass(NPAD, NBLK, F, GCAPS, HALF, GCHUNK)
        nc_.compile()
        _NC_CACHE["nc"] = nc_
    return _NC_CACHE["nc"]


def kernel(emb, edge_index, W1, b1, W2, b2):
    emb = np.asarray(emb)
    edge_index = np.asarray(edge_index)
    W1, b1, W2, b2 = (np.asarray(a) for a in (W1, b1, W2, b2))
    col = edge_index[0].astype(np.int64)
    row = edge_index[1].astype(np.int64)

    nc = get_nc()
    in_maps, scatter = make_inputs(emb, W1, b1, W2, b2, col, row, NPAD, GCAPS, NCORES)
    res = run_bass_kernel_spmd(nc, in_maps, core_ids=list(range(NCORES)))
    return unshard_output(res.results, scatter, GCAPS, EPC, NCORES).astype(np.float32)



# revision 8
# speedup vs baseline: 1.3174x; 1.3174x over previous
"""Trainium2 Bass kernel for nn_ExtractorMLP (GNN edge cosine-similarity logits).

Math: out[e] = cos(MLP(emb[col[e]]), MLP(emb[row[e]])) for E edges, where
MLP(x) = relu(x @ W1.T + b1) @ W2.T + b2, cos uses torch eps=1e-8 semantics.

Strategy (8 cores, SPMD, identical program, per-core edge shards):
  Phase 1 (pair-split): cores 2k/2k+1 each run the node MLP over HALF the
    nodes and write normalized bf16 rows into a pair-Shared DRAM table
    gn[N, H] (TRN2 pairs share an HBM domain, so both cores see one
    physical buffer). A pairwise AllReduce barrier orders table writes
    before any gather. Normalize path: DVE squares (bf16) + reduce,
    ACT per-chunk scale-copy with per-partition inv-norm.
  Phase 2 (edge shard, E/8 per core): chunked dma_gather of gn rows for
    col/row endpoints (edges across partitions, H contiguous), bf16
    elementwise multiply + free-axis reduce on DVE giving per-edge dots.
    dma_gather uses int16 indices, so edges are grouped host-side into 4
    groups by (col<32768, row<32768) and gathered against per-half table
    base offsets.
"""

import sys

for _p in ("/opt/trn_rl_repo",):
    if _p not in sys.path:
        sys.path.insert(0, _p)

import numpy as np
import ml_dtypes

import concourse.bass as bass
import concourse.bacc as bacc
import concourse.mybir as mybir
import concourse.tile as tile
from concourse.bass import ts
from concourse.tile import add_dep_helper
from concourse.bass_utils import run_bass_kernel_spmd

BF16 = mybir.dt.bfloat16
F32 = mybir.dt.float32
I16 = mybir.dt.int16

# Problem sizes (hardcoded per harness contract)
N, H, E = 50000, 256, 300000
NCORES = 8
F = 512                          # node-phase free-dim block (nodes per block)
NPAD = ((N + F - 1) // F) * F    # 50176
HALFN = NPAD // 2                # 25088 nodes per core (pair-split)
NBLKH = HALFN // F               # 49 blocks per core
EPC = E // NCORES                # 37500 edges per core
HALF = 32768                     # int16 index range split point
GCHUNK = 4096                    # edges per dma_gather


def _group_caps(epc):
    """Compile-time per-group capacities: mean + 8 sigma, rounded to 128."""
    p = HALF / N
    probs = [p * p, p * (1 - p), (1 - p) * p, (1 - p) * (1 - p)]
    caps = []
    for pr in probs:
        mean = epc * pr
        sig = (epc * pr * (1 - pr)) ** 0.5
        caps.append(int(np.ceil((mean + 8 * sig) / 128)) * 128)
    return caps


GCAPS = _group_caps(EPC)         # [16896, 9216, 9216, 5120]
GOFFS = [int(x) for x in np.cumsum([0] + GCAPS[:-1])]
TOTE = sum(GCAPS)                # 40448


def build_bass(n_pad, n_blk_half, f, gcaps, half, gchunk, table_dt=BF16):
    """Build the SPMD Bass module. Parametrized for small-scale sim tests."""
    nc = bacc.Bacc("TRN2", target_bir_lowering=False, num_swdge_queues=4)
    h = H
    half_n = n_pad // 2
    tote = sum(gcaps)
    goffs = [int(x) for x in np.cumsum([0] + list(gcaps[:-1]))]

    embT = nc.dram_tensor("embT", [h, half_n], BF16, kind="ExternalInput")
    w1t = nc.dram_tensor("w1t", [h, h], BF16, kind="ExternalInput")
    w2t = nc.dram_tensor("w2t", [h, h], BF16, kind="ExternalInput")
    b1c = nc.dram_tensor("b1c", [h, 1], F32, kind="ExternalInput")
    b2r2 = nc.dram_tensor("b2r2", [1, 2 * h], BF16, kind="ExternalInput")
    colw = nc.dram_tensor("colw", [128, tote // 16], I16, kind="ExternalInput")
    roww = nc.dram_tensor("roww", [128, tote // 16], I16, kind="ExternalInput")
    dots_out = nc.dram_tensor("dots", [128, tote // 128], F32, kind="ExternalOutput")
    gn = nc.dram_tensor("gn_table", [n_pad, h], table_dt, addr_space="Shared")
    ccin = nc.dram_tensor("ccin", [1, 8], F32)
    ccout = nc.dram_tensor("ccout", [1, 8], F32)

    AF = mybir.ActivationFunctionType
    OP = mybir.AluOpType
    AX = mybir.AxisListType

    with tile.TileContext(nc) as tc:
        with (
            tc.tile_pool(name="const", bufs=1) as constp,
            tc.tile_pool(name="xt", bufs=4) as xtp,
            tc.tile_pool(name="h1", bufs=3) as h1p,
            tc.tile_pool(name="gg", bufs=3) as gp,
            tc.tile_pool(name="small", bufs=4) as sp,
            tc.tile_pool(name="ps1", bufs=3, space="PSUM") as ps1,
            tc.tile_pool(name="ps2", bufs=2, space="PSUM") as ps2,
            tc.tile_pool(name="ebuf", bufs=3) as ep,
            tc.tile_pool(name="pbuf", bufs=2) as pp,
        ):
            # ---- constants ----
            w1k = []
            w2k = []
            b1t = []
            for k in range(2):
                t_ = constp.tile([128, h], BF16, tag=f"w1_{k}")
                nc.sync.dma_start(out=t_[:], in_=w1t[k * 128:(k + 1) * 128, :])
                w1k.append(t_)
                t_ = constp.tile([128, h], BF16, tag=f"w2_{k}")
                nc.sync.dma_start(out=t_[:], in_=w2t[k * 128:(k + 1) * 128, :])
                w2k.append(t_)
                t_ = constp.tile([128, 1], F32, tag=f"b1_{k}")
                nc.sync.dma_start(out=t_[:], in_=b1c[k * 128:(k + 1) * 128, :])
                b1t.append(t_)
            b2row = constp.tile([1, 2 * h], BF16, tag="b2row")
            nc.sync.dma_start(out=b2row[:], in_=b2r2[:])
            ones_row = constp.tile([1, 128], BF16, tag="ones_row")
            nc.vector.memset(ones_row[:], 1.0)
            colsb = constp.tile([128, tote // 16], I16, tag="colsb")
            nc.sync.dma_start(out=colsb[:], in_=colw[:])
            rowsb = constp.tile([128, tote // 16], I16, tag="rowsb")
            nc.sync.dma_start(out=rowsb[:], in_=roww[:])

            # which half of the shared table this core writes
            parity = nc.sync.partition_id() & 1

            # ---- phase 1: node MLP over this core's half -> shared table ----
            for b in range(n_blk_half):
                n0 = b * f
                xtk = []
                for k in range(2):
                    t_ = xtp.tile([128, f], BF16, tag=f"xt{k}")
                    nc.sync.dma_start(
                        out=t_[:], in_=embT[k * 128:(k + 1) * 128, n0:n0 + f]
                    )
                    xtk.append(t_)
                h1 = []
                for t in range(2):
                    p1 = ps1.tile([128, f], F32, tag="p1")
                    for k in range(2):
                        nc.tensor.matmul(
                            p1[:],
                            lhsT=w1k[k][:, t * 128:(t + 1) * 128],
                            rhs=xtk[k][:],
                            start=(k == 0),
                            stop=(k == 1),
                        )
                    ht = h1p.tile([128, f], BF16, tag=f"h1_{t}")
                    nc.scalar.activation(ht[:], p1[:], AF.Relu, bias=b1t[t][:])
                    h1.append(ht)
                nch = f // 128
                p2b = ps2.tile([128, nch, h], F32, tag="p2")
                for c in range(nch):
                    for t in range(2):
                        nc.tensor.matmul(
                            p2b[:, c, :],
                            lhsT=h1[t][:, c * 128:(c + 1) * 128],
                            rhs=w2k[t][:],
                            start=(t == 0),
                            stop=False,
                        )
                    # + b2 broadcast via rank-1 ones matmul (PE, not DVE)
                    nc.tensor.matmul(
                        p2b[:, c, :],
                        lhsT=ones_row[:],
                        rhs=b2row[:, :h],
                        start=False,
                        stop=True,
                    )
                # norms^2: ACT squares to bf16, then DVE free-axis reduce
                sqb = gp.tile([128, nch, h], BF16, tag="sqb")
                nc.scalar.activation(sqb[:], p2b[:], AF.Square)
                n2 = sp.tile([128, nch], F32, tag="n2")
                nc.vector.tensor_reduce(
                    out=n2[:], in_=sqb[:], axis=AX.X, op=OP.add,
                )
                s_ = sp.tile([128, nch], F32, tag="s")
                nc.scalar.activation(s_[:], n2[:], AF.Sqrt)
                sm = sp.tile([128, nch], F32, tag="sm")
                nc.vector.tensor_scalar_max(sm[:], s_[:], 1e-8)
                inv = sp.tile([128, nch], F32, tag="inv")
                nc.vector.reciprocal(inv[:], sm[:])
                gnb = gp.tile([128, nch, h], table_dt, tag="gnb")
                nc.vector.tensor_tensor(
                    out=gnb[:], in0=p2b[:],
                    in1=inv[:].to_broadcast([128, nch, h]), op=OP.mult,
                )
                nc.sync.dma_start(
                    out=gn[ts(parity * n_blk_half + b, f), :].rearrange(
                        "(c p) h -> p c h", p=128
                    ),
                    in_=gnb[:],
                )

            # all table writes (both pair halves) must land before any gather
            tc.strict_bb_all_engine_barrier()
            ccsb = constp.tile([1, 8], F32, tag="ccsb")
            nc.vector.memset(ccsb[:], 1.0)
            nc.sync.dma_start(out=ccin[:], in_=ccsb[:])
            nc.gpsimd.collective_compute(
                "AllReduce", mybir.AluOpType.add,
                replica_groups=[[0, 1], [2, 3], [4, 5], [6, 7]],
                ins=[ccin[:]], outs=[ccout[:]],
            )
            tc.strict_bb_all_engine_barrier()

            # ---- phase 2: grouped gathers (4 SWDGE queues) + dots ----
            bases = [
                (0, 0), (0, half), (half, 0), (half, half)
            ]  # (col base, row base) per group
            dots = constp.tile([128, tote // 128], F32, tag="dots")
            qi = 0
            prev_gather = None
            for g in range(4):
                cb, rb = bases[g]
                cb = cb if cb < n_pad else 0  # small-config: high groups empty
                rb = rb if rb < n_pad else 0
                src_c = gn[cb:, :] if cb else gn[:]
                src_r = gn[rb:, :] if rb else gn[:]
                for c0 in range(0, gcaps[g], gchunk):
                    nI = min(gchunk, gcaps[g] - c0)
                    nb = nI // 128
                    w0 = (goffs[g] + c0) // 16
                    g1 = ep.tile([128, nb, h], table_dt, tag="g1")
                    g2 = ep.tile([128, nb, h], table_dt, tag="g2")
                    gi1 = nc.gpsimd.dma_gather(
                        g1[:], src_c, colsb[:, w0:w0 + nI // 16],
                        nI, nI, h, transpose=False, single_packet=False,
                        queue_num=qi % 4,
                    )
                    qi += 1
                    gi2 = nc.gpsimd.dma_gather(
                        g2[:], src_r, rowsb[:, w0:w0 + nI // 16],
                        nI, nI, h, transpose=False, single_packet=False,
                        queue_num=qi % 4,
                    )
                    qi += 1
                    # pin scheduler order so DMASW lane rotation stays
                    # aligned with the queue_num stripe (lane i%8 <-> queue i%4)
                    if prev_gather is not None:
                        add_dep_helper(gi1.ins, prev_gather.ins, sync=False,
                                       reason="swdge lane/queue alignment")
                    add_dep_helper(gi2.ins, gi1.ins, sync=False,
                                   reason="swdge lane/queue alignment")
                    prev_gather = gi2
                    prod = pp.tile([128, nb, h], table_dt, tag="prod")
                    nc.vector.tensor_tensor(
                        out=prod[:], in0=g1[:], in1=g2[:], op=OP.mult,
                    )
                    b0 = (goffs[g] + c0) // 128
                    nc.vector.tensor_reduce(
                        out=dots[:, b0:b0 + nb], in_=prod[:], axis=AX.X, op=OP.add,
                    )
            nc.sync.dma_start(out=dots_out[:], in_=dots[:])

    return nc


def make_inputs(emb, W1, b1, W2, b2, col, row, n_pad, gcaps, ncores):
    """Host-side prep: transposes, bf16 rounding, per-core group shards.

    Returns (in_maps, scatter) where scatter[c] = (positions, goffs_lens)
    for reassembling per-core outputs.
    """
    h = emb.shape[1]
    half_n = n_pad // 2
    embT = np.zeros((h, n_pad), dtype=ml_dtypes.bfloat16)
    embT[:, :emb.shape[0]] = emb.astype(ml_dtypes.bfloat16).T
    embT_halves = [
        np.ascontiguousarray(embT[:, :half_n]),
        np.ascontiguousarray(embT[:, half_n:]),
    ]
    w1t = np.ascontiguousarray(W1.astype(ml_dtypes.bfloat16).T)
    w2t = np.ascontiguousarray(W2.astype(ml_dtypes.bfloat16).T)
    b1c = np.ascontiguousarray(b1.astype(np.float32).reshape(h, 1))
    b2r2 = np.tile(b2.astype(ml_dtypes.bfloat16).reshape(1, h), (1, 2))
    epc = len(col) // ncores
    goffs = [int(x) for x in np.cumsum([0] + list(gcaps[:-1]))]
    tote = sum(gcaps)

    def wrap16(a):
        return np.tile(a.reshape(-1, 16).T, (8, 1)).astype(np.int16)

    in_maps = []
    scatter = []
    for c in range(ncores):
        cs = col[c * epc:(c + 1) * epc].astype(np.int64)
        rs = row[c * epc:(c + 1) * epc].astype(np.int64)
        gid = (cs >= HALF) * 2 + (rs >= HALF)
        colw = np.zeros(tote, dtype=np.int16)
        roww = np.zeros(tote, dtype=np.int16)
        positions = []
        lens = []
        for g in range(4):
            pos = np.nonzero(gid == g)[0]
            pos = pos[np.argsort(cs[pos], kind="stable")]
            ng = len(pos)
            assert ng <= gcaps[g], f"group {g} overflow: {ng} > {gcaps[g]}"
            cb = HALF if g >= 2 else 0
            rb = HALF if g % 2 else 0
            colw[goffs[g]:goffs[g] + ng] = (cs[pos] - cb).astype(np.int16)
            roww[goffs[g]:goffs[g] + ng] = (rs[pos] - rb).astype(np.int16)
            positions.append(pos)
            lens.append(ng)
        in_maps.append({
            "embT": embT_halves[c % 2], "w1t": w1t, "w2t": w2t, "b1c": b1c,
            "b2r2": b2r2, "colw": wrap16(colw), "roww": wrap16(roww),
        })
        scatter.append((positions, lens))
    return in_maps, scatter


def unshard_output(outs, scatter, gcaps, epc, ncores):
    goffs = [int(x) for x in np.cumsum([0] + list(gcaps[:-1]))]
    parts = []
    for c in range(ncores):
        dots = np.asarray(outs[c]["dots"]).T.reshape(-1)
        positions, lens = scatter[c]
        res = np.empty(epc, dtype=np.float32)
        for g in range(4):
            res[positions[g]] = dots[goffs[g]:goffs[g] + lens[g]]
        parts.append(res)
    return np.concatenate(parts)


_NC_CACHE = {}


def get_nc():
    if "nc" not in _NC_CACHE:
        nc_ = build_bass(NPAD, NBLKH, F, GCAPS, HALF, GCHUNK)
        nc_.compile()
        _NC_CACHE["nc"] = nc_
    return _NC_CACHE["nc"]


def kernel(emb, edge_index, W1, b1, W2, b2):
    emb = np.asarray(emb)
    edge_index = np.asarray(edge_index)
    W1, b1, W2, b2 = (np.asarray(a) for a in (W1, b1, W2, b2))
    col = edge_index[0].astype(np.int64)
    row = edge_index[1].astype(np.int64)

    nc = get_nc()
    in_maps, scatter = make_inputs(emb, W1, b1, W2, b2, col, row, NPAD, GCAPS, NCORES)
    res = run_bass_kernel_spmd(nc, in_maps, core_ids=list(range(NCORES)))
    return unshard_output(res.results, scatter, GCAPS, EPC, NCORES).astype(np.float32)
